# revision 6
# baseline (speedup 1.0000x reference)
"""Trainium2 Bass kernel for ConditionalExpertRouter (dense MoE, all experts).

Math (per reference):
    rh    = relu(condition @ Wr1.T + br1)                  # [B, RH]
    route = softmax(rh @ Wr2.T + br2, axis=-1)             # [B, E]
    h_e   = relu(x @ W1[e].T + b1[e])                      # [B, H]
    y_e   = h_e @ W2[e].T + b2[e]                          # [B, D]
    out   = sum_e route[:, e] * y_e                        # [B, D]

Strategy: data-parallel over B across 8 cores (weights replicated).
On-chip layout is feature-major ("transposed"): activations live as
[feature(partitions), batch(free)] tiles so both expert matmuls contract
along the partition axis with zero on-chip transposes.  The softmax-
weighted sum over experts is folded into the second matmul's PSUM
accumulation: h'_e = relu(h_e) * exp_e, out_pre = sum_e W2[e].T @ h'_e
(+ b2.T @ exp matmul for the bias term), then one multiply by
1/sum_e exp_e.

v2 schedule (vs v1): router for both batch tiles runs up front while
x/W1/W2 stream in; the per-expert exp broadcast moved off the PE onto
the GpSimd engine (partition_broadcast); W2 is fully SBUF-resident
(loaded once, interleaved with W1); phase C runs dt-outer so PSUM
drains stagger and the end-of-kernel tail is one tile deep.

Expert matmuls run in bf16 (fp32 accumulation in PSUM); the router runs
in fp32.  Host-side prep does only layout transforms + dtype casts; all
model math happens on-device.
"""

import numpy as np
import ml_dtypes
from contextlib import ExitStack

import concourse.tile as tile
from concourse import bacc, mybir
from concourse.bass_utils import run_bass_kernel_spmd

BF16 = ml_dtypes.bfloat16

# Problem shapes (hardcoded per contract).
B, D, C, E, H, RH = 8192, 1024, 64, 16, 256, 128
NCORES = 8
BS = B // NCORES          # batch rows per core = 1024
NB = 512                  # batch tile (PSUM free-dim limit for fp32)
NBT = BS // NB            # batch tiles per core = 2
P = 128
KD = D // P               # k-tiles over D = 8
HT = H // P               # h-tiles over H = 2
DT = D // P               # d-tiles over D = 8

F32 = mybir.dt.float32
BF = mybir.dt.bfloat16
AF = mybir.ActivationFunctionType

_CACHE = {}


def _build():
    nc = bacc.Bacc("TRN2", target_bir_lowering=False, debug=False,
                   enable_asserts=False, num_devices=NCORES)

    # --- DRAM tensors (per-core) ---
    # xtp[p, kt*BS + b] = x[b, kt*128 + p]  (one big-descriptor DMA per kt)
    xtp = nc.dram_tensor("xtp", [P, KD * BS], BF, kind="ExternalInput").ap()
    condt = nc.dram_tensor("condt", [P, BS], F32, kind="ExternalInput").ap()
    # W1 expert-major: w1p[e, p, (ht*KD + kt)*P + hh] = W1[e, ht*P+hh, kt*P+p]
    w1p = nc.dram_tensor("w1p", [E, P, KD * H], BF, kind="ExternalInput").ap()
    w2p = nc.dram_tensor("w2p", [E, HT, P, D], BF, kind="ExternalInput").ap()
    # aux fp32 pack: [wr1p(128) | wr2t(16) | br1(1) | b1(32) | br2(1)] = 178
    auxp = nc.dram_tensor("auxp", [P, 178], F32, kind="ExternalInput").ap()
    b2p = nc.dram_tensor("b2p", [P, D], BF, kind="ExternalInput").ap()
    outt = nc.dram_tensor("outt", [D, BS], F32, kind="ExternalOutput").ap()

    with tile.TileContext(nc) as tc, ExitStack() as ctx:
        wp = ctx.enter_context(tc.tile_pool(name="resident", bufs=1))
        hpp = ctx.enter_context(tc.tile_pool(name="hprime", bufs=1))
        work = ctx.enter_context(tc.tile_pool(name="work", bufs=1))
        hrp = ctx.enter_context(tc.tile_pool(name="hrelu", bufs=4))
        bcp = ctx.enter_context(tc.tile_pool(name="bcast", bufs=3))
        scp = ctx.enter_context(tc.tile_pool(name="scratch", bufs=2))
        outp = ctx.enter_context(tc.tile_pool(name="outs", bufs=2))
        psA = ctx.enter_context(tc.tile_pool(name="psA", bufs=2, space="PSUM"))
        psB = ctx.enter_context(tc.tile_pool(name="psB", bufs=2, space="PSUM"))
        psC = ctx.enter_context(tc.tile_pool(name="psC", bufs=3, space="PSUM"))

        # --- DMA issue order (queues drain roughly proportionally, so the
        # early-needed set goes first): x, W1[0], router inputs, then the
        # W1 stream interleaved with the (fully resident) W2 stream. ---
        xtall = wp.tile([P, KD * BS], BF, tag="xt")
        for kt in range(KD):
            nc.sync.dma_start(xtall[:, kt * BS:(kt + 1) * BS],
                              xtp[:, kt * BS:(kt + 1) * BS])
        xtsb = [xtall[:, kt * BS:(kt + 1) * BS] for kt in range(KD)]

        hw = KD * P                      # columns per ht half of one expert
        w1sb = []
        for e in range(E):
            w1sb.append(wp.tile([P, KD * H], BF, tag=f"w1_{e}",
                                name=f"w1sb{e}"))

        def load_w1(e):
            for ht in range(HT):
                nc.sync.dma_start(w1sb[e][:, ht * hw:(ht + 1) * hw],
                                  w1p[e, :, ht * hw:(ht + 1) * hw])

        w2sb = [[None] * HT for _ in range(E)]

        def load_w2(e):
            for ht in range(HT):
                t = wp.tile([P, D], BF, tag=f"w2_{e}_{ht}",
                            name=f"w2sb{e}_{ht}")
                nc.sync.dma_start(t[:], w2p[e][ht][:])
                w2sb[e][ht] = t

        load_w1(0)
        condsb = wp.tile([P, BS], F32, tag="cond")
        nc.sync.dma_start(condsb[:], condt[:])
        auxsb = wp.tile([P, 178], F32, tag="aux")
        nc.sync.dma_start(auxsb[:], auxp[:])
        load_w1(1)
        b2sb = wp.tile([P, D], BF, tag="b2")
        nc.sync.dma_start(b2sb[:], b2p[:])
        for e in range(2, E):
            load_w1(e)
            load_w2(e - 2)
        load_w2(E - 2)
        load_w2(E - 1)

        wr1sb = auxsb[:, 0:P]
        wr2sb = auxsb[:, P:P + E]
        br1sb = auxsb[:, P + E:P + E + 1]
        b1sb = auxsb[:, P + E + 1:P + E + 1 + E * HT]
        br2sb = auxsb[:E, P + E + 1 + E * HT:P + E + 2 + E * HT]

        # sum-over-experts selector: ones on partitions 0..E-1
        sumsel = wp.tile([P, P], BF, tag="sumsel")
        nc.vector.memset(sumsel[:], 0.0)
        nc.vector.memset(sumsel[:E, :], 1.0)

        # --- PE warm-up: keep the PE busy (and the HAM clock ramping)
        # while the router inputs and x/W1 stream in. ---
        warm = wp.tile([P, 256], BF, tag="warm")
        nc.vector.memset(warm[:], 1.0)
        ps_w = psA.tile([P, 256], F32, tag="pa", name="ps_warm")
        for _ in range(16):
            nc.tensor.matmul(ps_w[:], lhsT=warm[:, 0:P], rhs=warm[:],
                             start=True, stop=True)

        # --- router for BOTH batch tiles up front ---
        expt = []
        recip = []
        for bt in range(NBT):
            bsl = slice(bt * NB, (bt + 1) * NB)
            ps_rh = psA.tile([P, NB], F32, tag="pa", name=f"ps_rh{bt}")
            nc.tensor.matmul(ps_rh[:], lhsT=wr1sb[:], rhs=condsb[:, bsl],
                             start=True, stop=True)
            rh_sb = work.tile([P, NB], F32, tag=f"rh{bt}", name=f"rh_sb{bt}")
            nc.scalar.activation(rh_sb[:], ps_rh[:], AF.Relu,
                                 bias=br1sb[:, 0:1])
            ps_lg = psA.tile([E, NB], F32, tag="pa", name=f"ps_lg{bt}")
            nc.tensor.matmul(ps_lg[:], lhsT=wr2sb[:], rhs=rh_sb[:],
                             start=True, stop=True)
            # exp(logits + br2) into zero-padded [128, NB] bf16 tile
            et = work.tile([P, NB], BF, tag=f"expt{bt}", name=f"expt{bt}")
            nc.vector.memset(et[:], 0.0)
            nc.scalar.activation(et[:E, :], ps_lg[:], AF.Exp,
                                 bias=br2sb[:, 0:1])
            ps_sum = psA.tile([P, NB], F32, tag="pa", name=f"ps_sum{bt}")
            nc.tensor.matmul(ps_sum[:], lhsT=sumsel[:], rhs=et[:],
                             start=True, stop=True)
            rc = work.tile([P, NB], F32, tag=f"recip{bt}", name=f"recip{bt}")
            nc.vector.reciprocal(rc[:], ps_sum[:])
            expt.append(et)
            recip.append(rc)

        for bt in range(NBT):
            bsl = slice(bt * NB, (bt + 1) * NB)

            # ---- phase B: h'_e = relu(W1[e] @ x + b1[e]) * exp_e ----
            # exp broadcast across partitions runs on GpSimd, off the PE.
            hp_big = hpp.tile([P, E * HT * NB], BF, tag="hp", name="hp_big")
            for e in range(E):
                # engine APs must start at partition 0/32/64; DMA has no such
                # restriction, so hop the exp row to partition 0 first.
                sc = scp.tile([1, NB], BF, tag="sc", name=f"sc{bt}_{e}")
                nc.sync.dma_start(sc[:], expt[bt][e:e + 1, :])
                bc = bcp.tile([P, NB], BF, tag="bc", name=f"bc{bt}_{e}")
                nc.gpsimd.partition_broadcast(bc[:], sc[0:1, :])
                for ht in range(HT):
                    j = e * HT + ht
                    ps_h = psB.tile([P, NB], F32, tag="ph", name=f"ps_h{j}")
                    for kt in range(KD):
                        col = (ht * KD + kt) * P
                        nc.tensor.matmul(ps_h[:],
                                         lhsT=w1sb[e][:, col:col + P],
                                         rhs=xtsb[kt][:, bsl],
                                         start=(kt == 0), stop=(kt == KD - 1))
                    hr = hrp.tile([P, NB], BF, tag="hr", name=f"hr{j}")
                    nc.scalar.activation(hr[:], ps_h[:], AF.Relu,
                                         bias=b1sb[:, j:j + 1])
                    nc.vector.tensor_mul(hp_big[:, j * NB:(j + 1) * NB],
                                         hr[:], bc[:])

            # ---- phase C: out[dt] = b2.T@exp + sum_e W2[e].T @ h'_e ----
            # dt-outer: each accumulator finishes 33 matmuls then drains
            # while the next one runs.
            for dt in range(DT):
                acc = psC.tile([P, NB], F32, tag="cacc", name=f"acc{bt}_{dt}")
                nc.tensor.matmul(acc[:], lhsT=b2sb[:, dt * P:(dt + 1) * P],
                                 rhs=expt[bt][:], start=True, stop=False)
                for e in range(E):
                    for ht in range(HT):
                        j = e * HT + ht
                        last = (e == E - 1 and ht == HT - 1)
                        nc.tensor.matmul(
                            acc[:],
                            lhsT=w2sb[e][ht][:, dt * P:(dt + 1) * P],
                            rhs=hp_big[:, j * NB:(j + 1) * NB],
                            start=False, stop=last)
                osb = outp.tile([P, NB], F32, tag="ot", name=f"ot{bt}_{dt}")
                nc.vector.tensor_mul(osb[:], acc[:], recip[bt][:])
                nc.sync.dma_start(outt[dt * P:(dt + 1) * P, bsl], osb[:])

    nc.compile()
    return nc


def _prep_shared(W1, b1, W2, b2, Wr1, br1, Wr2, br2):
    """Host-side layout transforms + casts for the (core-replicated) weights."""
    # w1p[e, p, (ht*KD + kt)*P + hh] = W1[e, ht*P + hh, kt*P + p]
    # (ht-major so each expert's W1 streams in per-ht halves)
    w1p = np.ascontiguousarray(
        W1.reshape(E, HT, P, KD, P).transpose(0, 4, 1, 3, 2)
        .reshape(E, P, KD * H)).astype(BF16)
    w2p = np.ascontiguousarray(
        W2.transpose(0, 2, 1).reshape(E, HT, P, D)).astype(BF16)
    # aux pack: [wr1p(128) | wr2t(16) | br1(1) | b1(32) | br2(1)]
    aux = np.zeros((P, 178), np.float32)
    aux[:C, 0:P] = Wr1.T                         # [C, RH], zero-padded K
    aux[:, P:P + E] = Wr2.T                      # [RH, E]
    aux[:, P + E] = br1                          # [RH]
    aux[:, P + E + 1:P + E + 1 + E * HT] = (
        b1.reshape(E, HT, P).transpose(2, 0, 1).reshape(P, E * HT))
    aux[:E, P + E + 1 + E * HT] = br2            # [E]
    b2p = np.zeros((P, D), BF16)
    b2p[:E, :] = b2.astype(BF16)
    return dict(w1p=w1p, w2p=w2p, auxp=aux, b2p=b2p)


LAST_RESULTS = None


def kernel(x, condition, W1, b1, W2, b2, Wr1, br1, Wr2, br2):
    global LAST_RESULTS
    if "nc" not in _CACHE:
        _CACHE["nc"] = _build()
    nc = _CACHE["nc"]

    shared = _prep_shared(W1, b1, W2, b2, Wr1, br1, Wr2, br2)
    xT = np.ascontiguousarray(x.astype(np.float32).T)        # [D, B]
    condT = np.zeros((P, B), np.float32)
    condT[:C, :] = condition.T

    in_maps = []
    for c in range(NCORES):
        sl = slice(c * BS, (c + 1) * BS)
        m = dict(shared)
        # xtp[p, kt*BS + b] = xT[kt*128 + p, b]
        m["xtp"] = np.ascontiguousarray(
            xT[:, sl].reshape(KD, P, BS).transpose(1, 0, 2).reshape(P, KD * BS)
        ).astype(BF16)
        m["condt"] = np.ascontiguousarray(condT[:, sl])
        in_maps.append(m)

    res = run_bass_kernel_spmd(nc, in_maps, core_ids=list(range(NCORES)))
    LAST_RESULTS = res

    out = np.empty((B, D), np.float32)
    for c in range(NCORES):
        out[c * BS:(c + 1) * BS, :] = res.results[c]["outt"].T
    return out


# revision 8
# speedup vs baseline: 1.0701x; 1.0701x over previous
"""Trainium2 Bass kernel for ConditionalExpertRouter (dense MoE, all experts).

Math (per reference):
    rh    = relu(condition @ Wr1.T + br1)                  # [B, RH]
    route = softmax(rh @ Wr2.T + br2, axis=-1)             # [B, E]
    h_e   = relu(x @ W1[e].T + b1[e])                      # [B, H]
    y_e   = h_e @ W2[e].T + b2[e]                          # [B, D]
    out   = sum_e route[:, e] * y_e                        # [B, D]

Strategy: data-parallel over B across 8 cores (weights replicated).
On-chip layout is feature-major ("transposed"): activations live as
[feature(partitions), batch(free)] tiles so both expert matmuls contract
along the partition axis with zero on-chip transposes.  The softmax-
weighted sum over experts is folded into the second matmul's PSUM
accumulation: h'_e = relu(h_e) * exp_e (exp replicated across partitions
via a one-hot selector matmul), out_pre = sum_e W2[e].T-matmuls of h'_e
(+ sum_e exp_e*b2[e]), then a single multiply by 1/sum_e exp_e.

v4 schedule (vs v1):
  - Router inputs (cond/aux/sel) are FIRST in DMA issue order and the
    router for BOTH batch tiles runs up front, so the PE does useful
    work while x/W1 stream in.
  - Phase C keeps the j-outer accumulation but splits the last batch
    tile's d-groups (4,3,1) so the final PSUM drains stagger and the
    end-of-kernel tail is one tile deep.

Expert matmuls run in bf16 (fp32 accumulation in PSUM); the router runs
in fp32.  Host-side prep does only layout transforms + dtype casts; all
model math happens on-device.
"""

import numpy as np
import ml_dtypes
from contextlib import ExitStack

import concourse.tile as tile
from concourse import bacc, mybir
from concourse.bass_utils import run_bass_kernel_spmd

BF16 = ml_dtypes.bfloat16

# Problem shapes (hardcoded per contract).
B, D, C, E, H, RH = 8192, 1024, 64, 16, 256, 128
NCORES = 8
BS = B // NCORES          # batch rows per core = 1024
NB = 512                  # batch tile (PSUM free-dim limit for fp32)
NBT = BS // NB            # batch tiles per core = 2
P = 128
KD = D // P               # k-tiles over D = 8
HT = H // P               # h-tiles over H = 2
DT = D // P               # d-tiles over D = 8

F32 = mybir.dt.float32
BF = mybir.dt.bfloat16
AF = mybir.ActivationFunctionType

_CACHE = {}


def _build():
    nc = bacc.Bacc("TRN2", target_bir_lowering=False, debug=False,
                   enable_asserts=False, num_devices=NCORES)

    # --- DRAM tensors (per-core) ---
    # xtp[p, kt*BS + b] = x[b, kt*128 + p]  (one big-descriptor DMA per kt)
    xtp = nc.dram_tensor("xtp", [P, KD * BS], BF, kind="ExternalInput").ap()
    condt = nc.dram_tensor("condt", [P, BS], F32, kind="ExternalInput").ap()
    # W1 expert-major: w1p[e, p, (ht*KD + kt)*P + hh] = W1[e, ht*P+hh, kt*P+p]
    w1p = nc.dram_tensor("w1p", [E, P, KD * H], BF, kind="ExternalInput").ap()
    w2p = nc.dram_tensor("w2p", [E, HT, P, D], BF, kind="ExternalInput").ap()
    # aux fp32 pack: [wr1p(128) | wr2t(16) | br1(1) | b1(32) | br2(1)] = 178
    auxp = nc.dram_tensor("auxp", [P, 178], F32, kind="ExternalInput").ap()
    b2p = nc.dram_tensor("b2p", [P, D], BF, kind="ExternalInput").ap()
    # selectors packed in SBUF layout: [128, (E+1)*128]
    selp = nc.dram_tensor("selp", [P, (E + 1) * P], BF, kind="ExternalInput").ap()
    outt = nc.dram_tensor("outt", [D, BS], F32, kind="ExternalOutput").ap()

    with tile.TileContext(nc) as tc, ExitStack() as ctx:
        wp = ctx.enter_context(tc.tile_pool(name="resident", bufs=1))
        w2s = ctx.enter_context(tc.tile_pool(name="w2s", bufs=12))
        hpp = ctx.enter_context(tc.tile_pool(name="hprime", bufs=2))
        work = ctx.enter_context(tc.tile_pool(name="work", bufs=1))
        hrp = ctx.enter_context(tc.tile_pool(name="hrelu", bufs=4))
        outp = ctx.enter_context(tc.tile_pool(name="outs", bufs=4))
        psA = ctx.enter_context(tc.tile_pool(name="psA", bufs=2, space="PSUM"))
        psB = ctx.enter_context(tc.tile_pool(name="psB", bufs=2, space="PSUM"))
        psC = ctx.enter_context(tc.tile_pool(name="psC", bufs=4, space="PSUM"))

        # --- DMA issue order: router inputs first (so the router can run
        # while x/W1 stream), then x, then W1 expert-by-expert. ---
        condsb = wp.tile([P, BS], F32, tag="cond")
        nc.sync.dma_start(condsb[:], condt[:])
        auxsb = wp.tile([P, 178], F32, tag="aux")
        nc.sync.dma_start(auxsb[:], auxp[:])
        selsb = wp.tile([P, (E + 1) * P], BF, tag="sel")
        nc.sync.dma_start(selsb[:], selp[:])

        xtall = wp.tile([P, KD * BS], BF, tag="xt")
        for kt in range(KD):
            nc.sync.dma_start(xtall[:, kt * BS:(kt + 1) * BS],
                              xtp[:, kt * BS:(kt + 1) * BS])
        xtsb = [xtall[:, kt * BS:(kt + 1) * BS] for kt in range(KD)]

        hw = KD * P                      # columns per ht half of one expert
        w1sb = []
        for e in range(E):
            w1sb.append(wp.tile([P, KD * H], BF, tag=f"w1_{e}",
                                name=f"w1sb{e}"))

        def load_w1(e):
            for ht in range(HT):
                nc.sync.dma_start(w1sb[e][:, ht * hw:(ht + 1) * hw],
                                  w1p[e, :, ht * hw:(ht + 1) * hw])

        load_w1(0)
        load_w1(1)
        b2sb = wp.tile([P, D], BF, tag="b2")
        nc.sync.dma_start(b2sb[:], b2p[:])
        for e in range(2, E):
            load_w1(e)

        wr1sb = auxsb[:, 0:P]
        wr2sb = auxsb[:, P:P + E]
        br1sb = auxsb[:, P + E:P + E + 1]
        b1sb = auxsb[:, P + E + 1:P + E + 1 + E * HT]
        br2sb = auxsb[:E, P + E + 1 + E * HT:P + E + 2 + E * HT]

        def sel_ap(s):
            return selsb[:, s * P:(s + 1) * P]

        # --- PE warm-up: keep the PE busy (and the HAM clock ramping)
        # while the router inputs stream in. ---
        warm = wp.tile([P, 256], BF, tag="warm")
        nc.vector.memset(warm[:], 1.0)
        ps_w = psA.tile([P, 256], F32, tag="pa", name="ps_warm")
        for _ in range(16):
            nc.tensor.matmul(ps_w[:], lhsT=warm[:, 0:P], rhs=warm[:],
                             start=True, stop=True)

        # --- router for BOTH batch tiles up front ---
        expt = []
        recip = []
        for bt in range(NBT):
            bsl = slice(bt * NB, (bt + 1) * NB)
            ps_rh = psA.tile([P, NB], F32, tag="pa", name=f"ps_rh{bt}")
            nc.tensor.matmul(ps_rh[:], lhsT=wr1sb[:], rhs=condsb[:, bsl],
                             start=True, stop=True)
            rh_sb = work.tile([P, NB], F32, tag=f"rh{bt}", name=f"rh_sb{bt}")
            nc.scalar.activation(rh_sb[:], ps_rh[:], AF.Relu,
                                 bias=br1sb[:, 0:1])
            ps_lg = psA.tile([E, NB], F32, tag="pa", name=f"ps_lg{bt}")
            nc.tensor.matmul(ps_lg[:], lhsT=wr2sb[:], rhs=rh_sb[:],
                             start=True, stop=True)
            # exp(logits + br2) into zero-padded [128, NB] bf16 tile
            et = work.tile([P, NB], BF, tag=f"expt{bt}", name=f"expt{bt}")
            nc.vector.memset(et[:], 0.0)
            nc.scalar.activation(et[:E, :], ps_lg[:], AF.Exp,
                                 bias=br2sb[:, 0:1])
            ps_sum = psA.tile([P, NB], F32, tag="pa", name=f"ps_sum{bt}")
            nc.tensor.matmul(ps_sum[:], lhsT=sel_ap(E), rhs=et[:],
                             start=True, stop=True)
            rc = work.tile([P, NB], F32, tag=f"recip{bt}", name=f"recip{bt}")
            nc.vector.reciprocal(rc[:], ps_sum[:])
            expt.append(et)
            recip.append(rc)

        for bt in range(NBT):
            bsl = slice(bt * NB, (bt + 1) * NB)

            # ---- phase B: h'_e = relu(W1[e] @ x + b1[e]) * exp_e ----
            hp_big = hpp.tile([P, E * HT * NB], BF, tag="hp", name="hp_big")
            for e in range(E):
                ps_rep = psA.tile([P, NB], F32, tag="pa", name=f"ps_rep{e}")
                nc.tensor.matmul(ps_rep[:], lhsT=sel_ap(e), rhs=expt[bt][:],
                                 start=True, stop=True)
                for ht in range(HT):
                    j = e * HT + ht
                    ps_h = psB.tile([P, NB], F32, tag="ph", name=f"ps_h{j}")
                    for kt in range(KD):
                        col = (ht * KD + kt) * P
                        nc.tensor.matmul(ps_h[:],
                                         lhsT=w1sb[e][:, col:col + P],
                                         rhs=xtsb[kt][:, bsl],
                                         start=(kt == 0), stop=(kt == KD - 1))
                    hr = hrp.tile([P, NB], BF, tag="hr", name=f"hr{j}")
                    nc.scalar.activation(hr[:], ps_h[:], AF.Relu,
                                         bias=b1sb[:, j:j + 1])
                    nc.vector.tensor_mul(hp_big[:, j * NB:(j + 1) * NB],
                                         hr[:], ps_rep[:])

            # ---- phase C: out[dt] = b2.T@exp + sum_e W2[e].T @ h'_e ----
            # j-outer within each d-group; last batch tile uses groups
            # (4,3,1) so the final drains stagger.
            groups = [4, 4] if bt < NBT - 1 else [4, 3, 1]
            dt0 = 0
            for gs in groups:
                accs = []
                for i in range(gs):
                    dt = dt0 + i
                    pa = psC.tile([P, NB], F32, tag="cacc",
                                  name=f"acc{bt}_{dt}")
                    nc.tensor.matmul(pa[:], lhsT=b2sb[:, dt * P:(dt + 1) * P],
                                     rhs=expt[bt][:], start=True, stop=False)
                    accs.append(pa)
                for e in range(E):
                    for ht in range(HT):
                        j = e * HT + ht
                        w2t = w2s.tile([P, gs * P], BF, tag="w2t",
                                       name=f"w2t{bt}_{dt0}_{j}")
                        nc.sync.dma_start(
                            w2t[:],
                            w2p[e][ht][:, dt0 * P:(dt0 + gs) * P])
                        last = (e == E - 1 and ht == HT - 1)
                        for i in range(gs):
                            nc.tensor.matmul(
                                accs[i][:], lhsT=w2t[:, i * P:(i + 1) * P],
                                rhs=hp_big[:, j * NB:(j + 1) * NB],
                                start=False, stop=last)
                for i in range(gs):
                    dt = dt0 + i
                    osb = outp.tile([P, NB], F32, tag="ot",
                                    name=f"ot{bt}_{dt}")
                    nc.vector.tensor_mul(osb[:], accs[i][:], recip[bt][:])
                    nc.sync.dma_start(outt[dt * P:(dt + 1) * P, bsl], osb[:])
                dt0 += gs

    nc.compile()
    return nc


def _prep_shared(W1, b1, W2, b2, Wr1, br1, Wr2, br2):
    """Host-side layout transforms + casts for the (core-replicated) weights."""
    # w1p[e, p, (ht*KD + kt)*P + hh] = W1[e, ht*P + hh, kt*P + p]
    # (ht-major so each expert's W1 streams in per-ht halves)
    w1p = np.ascontiguousarray(
        W1.reshape(E, HT, P, KD, P).transpose(0, 4, 1, 3, 2)
        .reshape(E, P, KD * H)).astype(BF16)
    w2p = np.ascontiguousarray(
        W2.transpose(0, 2, 1).reshape(E, HT, P, D)).astype(BF16)
    # aux pack: [wr1p(128) | wr2t(16) | br1(1) | b1(32) | br2(1)]
    aux = np.zeros((P, 178), np.float32)
    aux[:C, 0:P] = Wr1.T                         # [C, RH], zero-padded K
    aux[:, P:P + E] = Wr2.T                      # [RH, E]
    aux[:, P + E] = br1                          # [RH]
    aux[:, P + E + 1:P + E + 1 + E * HT] = (
        b1.reshape(E, HT, P).transpose(2, 0, 1).reshape(P, E * HT))
    aux[:E, P + E + 1 + E * HT] = br2            # [E]
    b2p = np.zeros((P, D), BF16)
    b2p[:E, :] = b2.astype(BF16)
    selp = np.zeros((P, (E + 1) * P), BF16)
    for e in range(E):
        selp[e, e * P:(e + 1) * P] = 1.0         # broadcast-row selector
    selp[:E, E * P:(E + 1) * P] = 1.0            # sum-over-experts selector
    return dict(w1p=w1p, w2p=w2p, auxp=aux, b2p=b2p, selp=selp)


LAST_RESULTS = None


def kernel(x, condition, W1, b1, W2, b2, Wr1, br1, Wr2, br2):
    global LAST_RESULTS
    if "nc" not in _CACHE:
        _CACHE["nc"] = _build()
    nc = _CACHE["nc"]

    shared = _prep_shared(W1, b1, W2, b2, Wr1, br1, Wr2, br2)
    xT = np.ascontiguousarray(x.astype(np.float32).T)        # [D, B]
    condT = np.zeros((P, B), np.float32)
    condT[:C, :] = condition.T

    in_maps = []
    for c in range(NCORES):
        sl = slice(c * BS, (c + 1) * BS)
        m = dict(shared)
        # xtp[p, kt*BS + b] = xT[kt*128 + p, b]
        m["xtp"] = np.ascontiguousarray(
            xT[:, sl].reshape(KD, P, BS).transpose(1, 0, 2).reshape(P, KD * BS)
        ).astype(BF16)
        m["condt"] = np.ascontiguousarray(condT[:, sl])
        in_maps.append(m)

    res = run_bass_kernel_spmd(nc, in_maps, core_ids=list(range(NCORES)))
    LAST_RESULTS = res

    out = np.empty((B, D), np.float32)
    for c in range(NCORES):
        out[c * BS:(c + 1) * BS, :] = res.results[c]["outt"].T
    return out


# revision 13
# speedup vs baseline: 1.0823x; 1.0114x over previous
"""Trainium2 Bass kernel for ConditionalExpertRouter (dense MoE, all experts).

Math (per reference):
    rh    = relu(condition @ Wr1.T + br1)                  # [B, RH]
    route = softmax(rh @ Wr2.T + br2, axis=-1)             # [B, E]
    h_e   = relu(x @ W1[e].T + b1[e])                      # [B, H]
    y_e   = h_e @ W2[e].T + b2[e]                          # [B, D]
    out   = sum_e route[:, e] * y_e                        # [B, D]

Strategy: data-parallel over B across 8 cores (weights replicated).
On-chip layout is feature-major ("transposed"): activations live as
[feature(partitions), batch(free)] tiles so both expert matmuls contract
along the partition axis with zero on-chip transposes.  The softmax-
weighted sum over experts is folded into the second matmul's PSUM
accumulation: h'_e = relu(h_e) * exp_e (exp replicated across partitions
via a one-hot selector matmul), out_pre = sum_e W2[e].T-matmuls of h'_e
(+ sum_e exp_e*b2[e]), then a single multiply by 1/sum_e exp_e.

v4 schedule (vs v1):
  - Router inputs (cond/aux/sel) are FIRST in DMA issue order and the
    router for BOTH batch tiles runs up front, so the PE does useful
    work while x/W1 stream in.
  - Phase C keeps the j-outer accumulation but splits the last batch
    tile's d-groups (4,3,1) so the final PSUM drains stagger and the
    end-of-kernel tail is one tile deep.

Expert matmuls run in bf16 (fp32 accumulation in PSUM); the router runs
in fp32.  Host-side prep does only layout transforms + dtype casts; all
model math happens on-device.
"""

import numpy as np
import ml_dtypes
from contextlib import ExitStack

import concourse.tile as tile
from concourse import bacc, mybir
from concourse.bass_utils import run_bass_kernel_spmd

BF16 = ml_dtypes.bfloat16

# Problem shapes (hardcoded per contract).
B, D, C, E, H, RH = 8192, 1024, 64, 16, 256, 128
NCORES = 8
BS = B // NCORES          # batch rows per core = 1024
NB = 512                  # batch tile (PSUM free-dim limit for fp32)
NBT = BS // NB            # batch tiles per core = 2
P = 128
KD = D // P               # k-tiles over D = 8
HT = H // P               # h-tiles over H = 2
DT = D // P               # d-tiles over D = 8

F32 = mybir.dt.float32
BF = mybir.dt.bfloat16
AF = mybir.ActivationFunctionType

_CACHE = {}


def _build():
    nc = bacc.Bacc("TRN2", target_bir_lowering=False, debug=False,
                   enable_asserts=False, num_devices=NCORES)

    # --- DRAM tensors (per-core) ---
    # xtp[p, kt*BS + b] = x[b, kt*128 + p]  (one big-descriptor DMA per kt)
    xtp = nc.dram_tensor("xtp", [P, KD * BS], BF, kind="ExternalInput").ap()
    condt = nc.dram_tensor("condt", [P, BS], F32, kind="ExternalInput").ap()
    # W1 expert-major: w1p[e, p, (ht*KD + kt)*P + hh] = W1[e, ht*P+hh, kt*P+p]
    w1p = nc.dram_tensor("w1p", [E, P, KD * H], BF, kind="ExternalInput").ap()
    w2p = nc.dram_tensor("w2p", [E, HT, P, D], BF, kind="ExternalInput").ap()
    # aux fp32 pack: [wr1p(128) | wr2t(16) | br1(1) | b1(32) | br2(1)] = 178
    auxp = nc.dram_tensor("auxp", [P, 178], F32, kind="ExternalInput").ap()
    b2p = nc.dram_tensor("b2p", [P, D], BF, kind="ExternalInput").ap()
    # selectors packed in SBUF layout: [128, (E+1)*128]
    selp = nc.dram_tensor("selp", [P, (E + 1) * P], BF, kind="ExternalInput").ap()
    outt = nc.dram_tensor("outt", [D, BS], F32, kind="ExternalOutput").ap()

    with tile.TileContext(nc) as tc, ExitStack() as ctx:
        wp = ctx.enter_context(tc.tile_pool(name="resident", bufs=1))
        w2s = ctx.enter_context(tc.tile_pool(name="w2s", bufs=12))
        hpp = ctx.enter_context(tc.tile_pool(name="hprime", bufs=2))
        work = ctx.enter_context(tc.tile_pool(name="work", bufs=1))
        hrp = ctx.enter_context(tc.tile_pool(name="hrelu", bufs=4))
        outp = ctx.enter_context(tc.tile_pool(name="outs", bufs=4))
        psA = ctx.enter_context(tc.tile_pool(name="psA", bufs=2, space="PSUM"))
        psB = ctx.enter_context(tc.tile_pool(name="psB", bufs=2, space="PSUM"))
        psC = ctx.enter_context(tc.tile_pool(name="psC", bufs=4, space="PSUM"))

        # --- DMA issue order: x first (it gates phase B and each issue
        # costs ~0.6us on the sync engine), then the router inputs, then
        # W1 expert-by-expert. ---
        xtall = wp.tile([P, KD * BS], BF, tag="xt")
        for kt in range(KD):
            nc.sync.dma_start(xtall[:, kt * BS:(kt + 1) * BS],
                              xtp[:, kt * BS:(kt + 1) * BS])
        xtsb = [xtall[:, kt * BS:(kt + 1) * BS] for kt in range(KD)]

        condsb = wp.tile([P, BS], F32, tag="cond")
        nc.sync.dma_start(condsb[:], condt[:])
        auxsb = wp.tile([P, 178], F32, tag="aux")
        nc.sync.dma_start(auxsb[:], auxp[:])
        selsb = wp.tile([P, (E + 1) * P], BF, tag="sel")
        nc.sync.dma_start(selsb[:], selp[:])

        hw = KD * P                      # columns per ht half of one expert
        w1sb = []
        for e in range(E):
            w1sb.append(wp.tile([P, KD * H], BF, tag=f"w1_{e}",
                                name=f"w1sb{e}"))

        def load_w1(e):
            for ht in range(HT):
                nc.sync.dma_start(w1sb[e][:, ht * hw:(ht + 1) * hw],
                                  w1p[e, :, ht * hw:(ht + 1) * hw])

        load_w1(0)
        load_w1(1)
        b2sb = wp.tile([P, D], BF, tag="b2")
        nc.sync.dma_start(b2sb[:], b2p[:])
        for e in range(2, E):
            load_w1(e)

        wr1sb = auxsb[:, 0:P]
        wr2sb = auxsb[:, P:P + E]
        br1sb = auxsb[:, P + E:P + E + 1]
        b1sb = auxsb[:, P + E + 1:P + E + 1 + E * HT]
        br2sb = auxsb[:E, P + E + 1 + E * HT:P + E + 2 + E * HT]

        def sel_ap(s):
            return selsb[:, s * P:(s + 1) * P]

        # --- PE warm-up: keep the PE busy (and the HAM clock ramping)
        # while the router inputs stream in. ---
        warm = wp.tile([P, NB], BF, tag="warm")
        nc.vector.memset(warm[:], 1.0)
        ps_w = psA.tile([P, NB], F32, tag="pa", name="ps_warm")
        for _ in range(16):
            nc.tensor.matmul(ps_w[:], lhsT=warm[:, 0:P], rhs=warm[:],
                             start=True, stop=True)

        # --- router for BOTH batch tiles up front ---
        expt = []
        recip = []
        for bt in range(NBT):
            bsl = slice(bt * NB, (bt + 1) * NB)
            ps_rh = psA.tile([P, NB], F32, tag="pa", name=f"ps_rh{bt}")
            nc.tensor.matmul(ps_rh[:], lhsT=wr1sb[:], rhs=condsb[:, bsl],
                             start=True, stop=True)
            rh_sb = work.tile([P, NB], F32, tag=f"rh{bt}", name=f"rh_sb{bt}")
            nc.scalar.activation(rh_sb[:], ps_rh[:], AF.Relu,
                                 bias=br1sb[:, 0:1])
            ps_lg = psA.tile([E, NB], F32, tag="pa", name=f"ps_lg{bt}")
            nc.tensor.matmul(ps_lg[:], lhsT=wr2sb[:], rhs=rh_sb[:],
                             start=True, stop=True)
            # exp(logits + br2) into zero-padded [128, NB] bf16 tile
            et = work.tile([P, NB], BF, tag=f"expt{bt}", name=f"expt{bt}")
            nc.vector.memset(et[:], 0.0)
            nc.scalar.activation(et[:E, :], ps_lg[:], AF.Exp,
                                 bias=br2sb[:, 0:1])
            ps_sum = psA.tile([P, NB], F32, tag="pa", name=f"ps_sum{bt}")
            nc.tensor.matmul(ps_sum[:], lhsT=sel_ap(E), rhs=et[:],
                             start=True, stop=True)
            rc = work.tile([P, NB], F32, tag=f"recip{bt}", name=f"recip{bt}")
            nc.vector.reciprocal(rc[:], ps_sum[:])
            expt.append(et)
            recip.append(rc)

        for bt in range(NBT):
            bsl = slice(bt * NB, (bt + 1) * NB)

            # ---- phase B: h'_e = relu(W1[e] @ x + b1[e]) * exp_e ----
            hp_big = hpp.tile([P, E * HT * NB], BF, tag="hp", name="hp_big")
            for e in range(E):
                ps_rep = psA.tile([P, NB], F32, tag="pa", name=f"ps_rep{e}")
                nc.tensor.matmul(ps_rep[:], lhsT=sel_ap(e), rhs=expt[bt][:],
                                 start=True, stop=True)
                for ht in range(HT):
                    j = e * HT + ht
                    ps_h = psB.tile([P, NB], F32, tag="ph", name=f"ps_h{j}")
                    for kt in range(KD):
                        col = (ht * KD + kt) * P
                        nc.tensor.matmul(ps_h[:],
                                         lhsT=w1sb[e][:, col:col + P],
                                         rhs=xtsb[kt][:, bsl],
                                         start=(kt == 0), stop=(kt == KD - 1))
                    hr = hrp.tile([P, NB], BF, tag="hr", name=f"hr{j}")
                    nc.scalar.activation(hr[:], ps_h[:], AF.Relu,
                                         bias=b1sb[:, j:j + 1])
                    nc.vector.tensor_mul(hp_big[:, j * NB:(j + 1) * NB],
                                         hr[:], ps_rep[:])

            # ---- phase C: out[dt] = b2.T@exp + sum_e W2[e].T @ h'_e ----
            groups = [4, 4]
            dt0 = 0
            for gi, gs in enumerate(groups):
                final = (bt == NBT - 1 and gi == len(groups) - 1)
                accs = []
                for i in range(gs):
                    dt = dt0 + i
                    pa = psC.tile([P, NB], F32, tag="cacc",
                                  name=f"acc{bt}_{dt}")
                    nc.tensor.matmul(pa[:], lhsT=b2sb[:, dt * P:(dt + 1) * P],
                                     rhs=expt[bt][:], start=True, stop=False)
                    accs.append(pa)
                for e in range(E):
                    for ht in range(HT):
                        j = e * HT + ht
                        w2t = w2s.tile([P, gs * P], BF, tag="w2t",
                                       name=f"w2t{bt}_{dt0}_{j}")
                        nc.sync.dma_start(
                            w2t[:],
                            w2p[e][ht][:, dt0 * P:(dt0 + gs) * P])
                        last = (e == E - 1 and ht == HT - 1)
                        for i in range(gs):
                            nc.tensor.matmul(
                                accs[i][:], lhsT=w2t[:, i * P:(i + 1) * P],
                                rhs=hp_big[:, j * NB:(j + 1) * NB],
                                start=False, stop=last)
                for i in range(gs):
                    dt = dt0 + i
                    osb = outp.tile([P, NB], F32, tag="ot",
                                    name=f"ot{bt}_{dt}")
                    if final:
                        # end-of-kernel tail: split each drain across the
                        # Vector and GpSimd engines and DMA the halves on
                        # separate rings so the last tile clears sooner.
                        hb = NB // 2
                        nc.vector.tensor_mul(osb[:, :hb], accs[i][:, :hb],
                                             recip[bt][:, :hb])
                        nc.vector.tensor_mul(osb[:, hb:], accs[i][:, hb:],
                                             recip[bt][:, hb:])
                        lo = slice(bt * NB, bt * NB + hb)
                        hi = slice(bt * NB + hb, (bt + 1) * NB)
                        nc.sync.dma_start(outt[dt * P:(dt + 1) * P, lo],
                                          osb[:, :hb])
                        nc.sync.dma_start(outt[dt * P:(dt + 1) * P, hi],
                                          osb[:, hb:])
                    else:
                        nc.vector.tensor_mul(osb[:], accs[i][:], recip[bt][:])
                        nc.sync.dma_start(outt[dt * P:(dt + 1) * P, bsl],
                                          osb[:])
                dt0 += gs

    nc.compile()
    return nc


def _prep_shared(W1, b1, W2, b2, Wr1, br1, Wr2, br2):
    """Host-side layout transforms + casts for the (core-replicated) weights."""
    # w1p[e, p, (ht*KD + kt)*P + hh] = W1[e, ht*P + hh, kt*P + p]
    # (ht-major so each expert's W1 streams in per-ht halves)
    w1p = np.ascontiguousarray(
        W1.reshape(E, HT, P, KD, P).transpose(0, 4, 1, 3, 2)
        .reshape(E, P, KD * H)).astype(BF16)
    w2p = np.ascontiguousarray(
        W2.transpose(0, 2, 1).reshape(E, HT, P, D)).astype(BF16)
    # aux pack: [wr1p(128) | wr2t(16) | br1(1) | b1(32) | br2(1)]
    aux = np.zeros((P, 178), np.float32)
    aux[:C, 0:P] = Wr1.T                         # [C, RH], zero-padded K
    aux[:, P:P + E] = Wr2.T                      # [RH, E]
    aux[:, P + E] = br1                          # [RH]
    aux[:, P + E + 1:P + E + 1 + E * HT] = (
        b1.reshape(E, HT, P).transpose(2, 0, 1).reshape(P, E * HT))
    aux[:E, P + E + 1 + E * HT] = br2            # [E]
    b2p = np.zeros((P, D), BF16)
    b2p[:E, :] = b2.astype(BF16)
    selp = np.zeros((P, (E + 1) * P), BF16)
    for e in range(E):
        selp[e, e * P:(e + 1) * P] = 1.0         # broadcast-row selector
    selp[:E, E * P:(E + 1) * P] = 1.0            # sum-over-experts selector
    return dict(w1p=w1p, w2p=w2p, auxp=aux, b2p=b2p, selp=selp)


LAST_RESULTS = None


def kernel(x, condition, W1, b1, W2, b2, Wr1, br1, Wr2, br2):
    global LAST_RESULTS
    if "nc" not in _CACHE:
        _CACHE["nc"] = _build()
    nc = _CACHE["nc"]

    shared = _prep_shared(W1, b1, W2, b2, Wr1, br1, Wr2, br2)
    xT = np.ascontiguousarray(x.astype(np.float32).T)        # [D, B]
    condT = np.zeros((P, B), np.float32)
    condT[:C, :] = condition.T

    in_maps = []
    for c in range(NCORES):
        sl = slice(c * BS, (c + 1) * BS)
        m = dict(shared)
        # xtp[p, kt*BS + b] = xT[kt*128 + p, b]
        m["xtp"] = np.ascontiguousarray(
            xT[:, sl].reshape(KD, P, BS).transpose(1, 0, 2).reshape(P, KD * BS)
        ).astype(BF16)
        m["condt"] = np.ascontiguousarray(condT[:, sl])
        in_maps.append(m)

    res = run_bass_kernel_spmd(nc, in_maps, core_ids=list(range(NCORES)))
    LAST_RESULTS = res

    out = np.empty((B, D), np.float32)
    for c in range(NCORES):
        out[c * BS:(c + 1) * BS, :] = res.results[c]["outt"].T
    return out
